# revision 26
# baseline (speedup 1.0000x reference)
"""Causal single-head attention on 8 trn2 NeuronCores.

Sharding: core c handles batch c//2 and half the query rows of that batch
(4 blocks of 256 rows, picked so causal work balances). The device program is
identical on every core; which rows a core owns is data (host-side
gather/scatter + per-core causal masks).

Algorithm (v2) — projections folded away:
  scores = x (Wq^T Wk) x^T and out = P x Wv, so the device never forms
  Q, K, or V:
    host:  A = Wq^T @ Wk  (f32)
    dev:   T^T = A^T x^T  over own queries            (xA)
           per query-block: S^T[j,i] = x^T.T_tiles @ T^T   (PSUM)
             + causal mask add, probsT = exp(S^T/32)  (no max needed:
               scaled scores are O(+-2))
           U^T[d,i] += x_nat_tile.T @ probsT   accumulated over j-tiles
           l[1,i]   += ones.T @ probsT         (softmax denominator)
           U^T /= l (broadcast) -> sbuf, then out = (U^T).T @ Wv^T tiles.
"""

import sys

try:
    import concourse  # noqa: F401
except ImportError:
    sys.path.insert(0, "/opt/trn_rl_repo")

from contextlib import ExitStack

import ml_dtypes
import numpy as np

import concourse.bass as bass
from concourse import bacc
import concourse.mybir as mybir
import concourse.tile as tile
from concourse.bass_utils import run_bass_kernel_spmd

B, N, D = 4, 2048, 1024
NQ = 1024            # query rows owned per core
NCORES = 8
TRIPS = (4, 8, 12, 16)          # j-tile trip count per slot (uniform program)
SLOTS = ((0, 2, 4, 6), (1, 3, 5, 7))  # 256-row block owned by slot s, per h
SCALE = 1.0 / 32.0   # 1/sqrt(D)
IB = 256             # query block width
MDT = mybir.dt.bfloat16
NPDT = ml_dtypes.bfloat16

TRACE = False
LAST_RESULT = None
LAST_IN_MAPS = None
_CACHED_NC = None


def _qrows(h):
    return np.concatenate([np.arange(256 * p, 256 * p + 256) for p in SLOTS[h]])


def _build_masks(h):
    """[4 slots, 4, 128, 256] f32: additive causal masks for the last 4 j-tiles
    of each slot (covers the diagonal tiles and the padded tiles)."""
    masks = np.zeros((4, 4, 128, IB), np.float32)
    jp = np.arange(128)[:, None]
    iv = np.arange(IB)[None, :]
    for s in range(4):
        r0 = 256 * SLOTS[h][s]
        for k in range(4):
            jt = TRIPS[s] - 4 + k
            masks[s, k] = np.where(jt * 128 + jp <= r0 + iv, 0.0, -1e30)
    return masks


def _build_body(nc, tc, ctx, dram, rep):
    P = 128
    n_d = D // P          # 8
    n_j = N // P          # 16
    xt_w = N + NQ
    xt_d, xn_d, a_d, wvt_d, mask_d, out_d = dram
    r = rep

    pool_xt = ctx.enter_context(tc.tile_pool(name=f"xt{r}", bufs=4 * n_d))
    pool_xq = ctx.enter_context(tc.tile_pool(name=f"xq{r}", bufs=n_d))
    pool_xn = ctx.enter_context(tc.tile_pool(name=f"xn{r}", bufs=n_j))
    pool_a = ctx.enter_context(tc.tile_pool(name=f"a{r}", bufs=n_d))
    pool_wv = ctx.enter_context(tc.tile_pool(name=f"wv{r}", bufs=n_d))
    pool_tt = ctx.enter_context(tc.tile_pool(name=f"tt{r}", bufs=n_d))
    pool_mask = ctx.enter_context(tc.tile_pool(name=f"mask{r}", bufs=16))
    pool_probs = ctx.enter_context(tc.tile_pool(name=f"probs{r}", bufs=16))
    pool_ut = ctx.enter_context(tc.tile_pool(name=f"ut{r}", bufs=2 * n_d))
    pool_lr = ctx.enter_context(tc.tile_pool(name=f"lr{r}", bufs=4))
    pool_out = ctx.enter_context(tc.tile_pool(name=f"outb{r}", bufs=2))
    pool_one = ctx.enter_context(tc.tile_pool(name=f"one{r}", bufs=1))

    # ---- loads ----
    ats = []
    for dt in range(n_d):
        t = pool_a.tile([P, D], MDT, tag="a", name=f"at{r}_{dt}")
        nc.scalar.dma_start(out=t, in_=a_d[dt * P:(dt + 1) * P, :])
        ats.append(t)
    xqs = []
    for dt in range(n_d):
        t = pool_xq.tile([P, NQ], MDT, tag="xq", name=f"xqt{r}_{dt}")
        nc.sync.dma_start(out=t, in_=xt_d[dt * P:(dt + 1) * P, N:])
        xqs.append(t)
    # key tiles [dt][jc]: [128, 512] each (4 j-chunks); DMA in first-use order
    xtk = [[None] * 4 for _ in range(n_d)]
    xns = [None] * n_j
    mask_tiles = [[None] * 4 for _ in range(4)]
    wvs = [None] * n_d

    def load_keys(jc):
        for dt in range(n_d):
            t = pool_xt.tile([P, 512], MDT, tag="xt", name=f"xtt{r}_{dt}_{jc}")
            nc.sync.dma_start(out=t, in_=xt_d[dt * P:(dt + 1) * P,
                                             jc * 512:(jc + 1) * 512])
            xtk[dt][jc] = t

    def load_xn(jc):
        for jt in range(4 * jc, 4 * jc + 4):
            t = pool_xn.tile([P, D], MDT, tag="xn", name=f"xnt{r}_{jt}")
            nc.sync.dma_start(out=t, in_=xn_d[jt * P:(jt + 1) * P, :])
            xns[jt] = t

    load_keys(0)
    load_xn(0)
    for s in range(4):
        for k in range(4):
            t = pool_mask.tile([P, IB], MDT, tag="mask",
                               name=f"mask{r}_{s}_{k}")
            nc.scalar.dma_start(out=t, in_=mask_d[s, k, :, :])
            mask_tiles[s][k] = t
    for dt in range(n_d):
        t = pool_wv.tile([P, D], MDT, tag="wv", name=f"wvt{r}_{dt}")
        nc.scalar.dma_start(out=t, in_=wvt_d[dt * P:(dt + 1) * P, :])
        wvs[dt] = t
    for jc in range(1, 4):
        load_keys(jc)
        load_xn(jc)
    ones = pool_one.tile([P, 1], MDT, tag="one", name=f"ones{r}")
    nc.vector.memset(ones, 1.0)

    # ---- phase 1: T^T[d2, i] = sum_d1 A[d1, d2] x^T[d1, i] over own queries
    tts = [pool_tt.tile([P, NQ], MDT, tag="tt", name=f"ttt{r}_{i}")
           for i in range(n_d)]
    with tc.tile_pool(name=f"ps1_{r}", bufs=4, space="PSUM") as ps1:
        for c0 in range(0, NQ, IB):
            for d2 in range(n_d):
                ps = ps1.tile([P, IB], mybir.dt.float32, tag="ps1",
                              name=f"pst{r}_{d2}_{c0}")
                for d1 in range(n_d):
                    nc.tensor.matmul(
                        ps,
                        lhsT=ats[d1][:, d2 * P:(d2 + 1) * P],
                        rhs=xqs[d1][:, c0:c0 + IB],
                        start=(d1 == 0), stop=(d1 == n_d - 1),
                    )
                nc.vector.tensor_copy(tts[d2][:, c0:c0 + IB], ps)

    # ---- phase 2: attention ----
    with (
        tc.tile_pool(name=f"ps_s{r}", bufs=2, space="PSUM") as ps_s,
        tc.tile_pool(name=f"ps_u{r}", bufs=2, space="PSUM") as ps_u,
        tc.tile_pool(name=f"ps_f{r}", bufs=2, space="PSUM") as ps_f,
        tc.tile_pool(name=f"ps_l{r}", bufs=2, space="PSUM") as ps_l,
    ):
        for s in range(4):
            trips = TRIPS[s]
            # pass 1: scores + exp; probs tiles persist for the slot
            probs_tiles = []
            for jt in range(trips):
                pss = ps_s.tile([P, IB], mybir.dt.float32, tag="pss",
                                name=f"pss{r}_{s}_{jt}")
                for d2 in range(n_d):
                    nc.tensor.matmul(
                        pss,
                        lhsT=xtk[d2][jt // 4][:, (jt % 4) * P:(jt % 4 + 1) * P],
                        rhs=tts[d2][:, s * IB:(s + 1) * IB],
                        start=(d2 == 0), stop=(d2 == n_d - 1),
                    )
                k = jt - (trips - 4)
                if k >= 0:
                    nc.vector.tensor_add(pss, pss, mask_tiles[s][k])
                probs = pool_probs.tile([P, IB], MDT, tag="probs",
                                        name=f"probs{r}_{s}_{jt}")
                nc.scalar.activation(probs, pss,
                                     mybir.ActivationFunctionType.Exp,
                                     scale=SCALE)
                probs_tiles.append(probs)

            # pass 2: U^T[d-tile, i] = sum_jt xn_tile.T @ probs, one bank
            # at a time so each accumulation group owns its bank exclusively.
            # The l^T (softmax denominator) matmuls are interleaved so their
            # LDWEIGHTS hide under the U streams in the PE reorder window.
            psl = [ps_l.tile([P, 1], mybir.dt.float32, tag="l",
                             name=f"psl{r}_{s}_{i}") for i in range(2)]
            uts = []
            for dt in range(n_d):
                psu = ps_u.tile([P, IB], mybir.dt.float32, tag="u",
                                name=f"psu{r}_{s}_{dt}")
                for jt in range(trips):
                    nc.tensor.matmul(
                        psu,
                        lhsT=xns[jt][:, dt * P:(dt + 1) * P],
                        rhs=probs_tiles[jt],
                        start=(jt == 0), stop=(jt == trips - 1),
                    )
                    if dt < 2:
                        nc.tensor.matmul(
                            psl[dt],
                            lhsT=probs_tiles[jt][:, dt * P:(dt + 1) * P],
                            rhs=ones,
                            start=(jt == 0), stop=(jt == trips - 1),
                        )
                ut = pool_ut.tile([P, IB], MDT, tag="ut",
                                  name=f"ut{r}_{s}_{dt}")
                nc.vector.tensor_copy(ut, psu)
                uts.append(ut)

            # out[i, o'] = (sum_d U^T[d, i]^T Wv^T[d, o']) / l[i]
            for half in range(2):
                rt = pool_lr.tile([P, 1], mybir.dt.float32, tag="lr",
                                  name=f"lrec{r}_{s}_{half}")
                nc.vector.reciprocal(rt, psl[half])
                obh = pool_out.tile([P, D], mybir.dt.float32, tag="obh",
                                    name=f"obh{r}_{s}_{half}")
                for c0 in range(0, D, 512):
                    psf = ps_f.tile([P, 512], mybir.dt.float32, tag="f",
                                    name=f"psf{r}_{s}_{half}_{c0}")
                    for dt in range(n_d):
                        nc.tensor.matmul(
                            psf,
                            lhsT=uts[dt][:, half * P:(half + 1) * P],
                            rhs=wvs[dt][:, c0:c0 + 512],
                            start=(dt == 0), stop=(dt == n_d - 1),
                        )
                    nc.vector.tensor_scalar_mul(obh[:, c0:c0 + 512], psf, rt)
                r0 = s * IB + half * P
                nc.sync.dma_start(out=out_d[r0:r0 + P, :], in_=obh)


def _build_nc(reps=1):
    nc = bacc.Bacc(None, target_bir_lowering=False)
    P = 128
    xt_w = N + NQ

    xt_d = nc.declare_dram_parameter("xt", [D, xt_w], MDT, isOutput=False)
    xn_d = nc.declare_dram_parameter("xn", [N, D], MDT, isOutput=False)
    a_d = nc.declare_dram_parameter("a", [D, D], MDT, isOutput=False)
    wvt_d = nc.declare_dram_parameter("wvt", [D, D], MDT, isOutput=False)
    mask_d = nc.declare_dram_parameter("masks", [4, 4, P, IB], MDT,
                                       isOutput=False)
    out_d = nc.declare_dram_parameter("out_p", [NQ, D], mybir.dt.float32,
                                      isOutput=True)
    dram = (xt_d, xn_d, a_d, wvt_d, mask_d, out_d)

    with tile.TileContext(nc) as tc:
        for rep in range(reps):
            with ExitStack() as ctx:
                _build_body(nc, tc, ctx, dram, rep)
    nc.finalize()
    return nc


def _make_in_maps(x, W_q, W_k, W_v):
    wq = np.asarray(W_q, np.float32)
    wk = np.asarray(W_k, np.float32)
    wv = np.asarray(W_v, np.float32)
    a = (wq.T @ wk).astype(NPDT)                       # [d1, d2]
    wvt = np.ascontiguousarray(wv.T).astype(NPDT)      # [d, o]
    masks = [_build_masks(0), _build_masks(1)]
    qrows = [_qrows(0), _qrows(1)]
    in_maps = []
    for c in range(NCORES):
        b, h = c // 2, c % 2
        xb = x[b]
        xb_t = xb.T  # [D, N]
        xt_all = np.concatenate([xb_t, xb_t[:, qrows[h]]], axis=1)
        in_maps.append({
            "xt": np.ascontiguousarray(xt_all).astype(NPDT),
            "xn": xb.astype(NPDT),
            "a": a, "wvt": wvt,
            "masks": masks[h].astype(NPDT),
        })
    return in_maps


def kernel(x, W_q, W_k, W_v):
    global _CACHED_NC, LAST_RESULT, LAST_IN_MAPS
    x = np.asarray(x, dtype=np.float32)
    if _CACHED_NC is None:
        _CACHED_NC = _build_nc()
    nc = _CACHED_NC

    in_maps = _make_in_maps(x, W_q, W_k, W_v)
    LAST_IN_MAPS = in_maps
    res = run_bass_kernel_spmd(nc, in_maps, list(range(NCORES)))
    LAST_RESULT = res

    qrows = [_qrows(0), _qrows(1)]
    out = np.empty((B, N, D), np.float32)
    for c in range(NCORES):
        b, h = c // 2, c % 2
        out[b, qrows[h], :] = res.results[c]["out_p"]
    return out
